# revision 31
# baseline (speedup 1.0000x reference)
"""Trainium2 Bass kernel for nn_AgentNetworkWHypernet (B=512, N=4).

Data-parallel over batch: 8 cores x 256 samples. Per sample the reference
generates a per-sample MLP from a hypernet embedding h = ReLU(ids@l1W.T+l1b)
and applies it to x. Per-sample weights are never materialized; per layer:

    C[bn,(m,e)] = x @ wW.reshape(S, M*E)     # shared-weight GEMM (TensorE, f32r)
    y[bn,m] = sum_e h[bn,e]*C[bn,m,e] + (x@wb + h_ib@bW.T + bb)[bn,m]

Stage 2 uses a custom fused DVE op: inclusive prefix-scan of (C * h_rep)
along the free dim; E-segment sums drop out as boundary-sample differences.
Bias path accumulates in PSUM via small matmuls (ones-row trick for bb).
Inter-layer activations are transposed on TensorE and ReLU'd on ScalarE on
the way back to SBUF as the next layer's stationary tiles.

Host-side prep is layout-only: batch sharding, reshapes, and transposes of
the small tensors (input shard, l1_W, bW) so the kernel has no startup
transpose chain. The big wW weights stream in natural layout.
"""
import numpy as np
from contextlib import ExitStack

import concourse.bass as bass
import concourse.mybir as mybir
import concourse.tile as tile
from concourse import bacc
from concourse.bass_utils import run_bass_kernel_spmd
import concourse.dve_ops as dve_ops
from concourse.dve_ops import DveOp
from concourse.dve_spec import Spec, Scan, AluOp, Src0, Src1

F32 = mybir.dt.float32
F32R = mybir.dt.float32r
AF = mybir.ActivationFunctionType
ALU = mybir.AluOpType

# Problem shapes (hardcoded per contract)
B, N = 512, 4
OBS, OBSU, ID = 128, 64, 16
STATE = OBS + OBSU - ID            # 176
M1, M2, G1, A = 256, 256, 256, 16
E = 128
NCORES = 8
BN = (B // NCORES) * N             # 256 samples per core
NBN = BN // 128                    # 2 bn-tiles

# layer: (S, M, relu, iw_chunk_index)
LAYERS = [
    (STATE, M1, True),
    (M1, M2, True),
    (M2, M2, True),
    (M2, G1, False),
    (G1, A, True),
]

CHUNK = 1024           # psum chunk of (m,e) columns = 8 m-groups (2 psum banks)
# Offload every OFFLOAD_MOD-th C chunk's h-contraction to ACT(evac) +
# Pool(mult + tree-reduce), freeing DVE scan cycles. 0 disables.
# Measured on HW: the Pool path is slower than modeled and regresses the
# kernel (+40us at MOD=7), so it stays off.
OFFLOAD_MOD = 0
# scan in1: stride-0 broadcast AP over h ("bcast") vs materialized hrep
# tiles ("mat", built on ScalarE)
HREP_BCAST = True
# engine for per-chunk boundary extraction (seg-0 copy + diffs)
BOUNDARY_ENG = "gpsimd"
REPEAT = 1             # benchmark knob: repeat whole computation in a For_i loop
PROBE = ""             # benchmark knob: "" | "dmaonly" | "nostage2" (set via module attr)
WDMA = 2048            # weight dma tile free elems (1 MB per 128-row tile)
WPOOL_BUFS = 12
# "f32r" | "bf16": dtype of the C-GEMM operands (wW weights, x stationaries,
# wb). neuronxcc rejects mixed 32/16-bit matmuls, so bf16 applies to both
# sides of the big GEMM; the small hypernet/bias-path matmuls stay f32r.
WW_DT = "bf16"


def ceil_div(a, b):
    return (a + b - 1) // b


def _ref_ttms(in0, in1, c0, c1, c2):
    return np.cumsum(in0.astype(np.float32) * in1.astype(np.float32), axis=-1)


def _make_scan_op():
    """Custom DVE op: inclusive prefix-sum of (in0*in1) along the free dim."""
    name = "TT_MUL_SCAN"
    if name in dve_ops._SUB_OPCODE_FOR_NAME:
        return next(o for o in dve_ops.OPS if o.name == name)
    spec = Spec(body=Scan(AluOp.ADD, Src0 * Src1), reference=_ref_ttms)
    dve_ops._SUB_OPCODE_FOR_NAME[name] = (
        dve_ops._CUSTOM_DVE_ROW_BASE + len(dve_ops.OPS)
    )
    op = DveOp(name, spec, subdim=False, uops_sha={"v3": "?", "v4": "?"})
    sha = None
    try:
        op.compile("v3")
    except ValueError as e:
        sha = str(e).split("(v3: ")[1].split(" ")[0]
    op = DveOp(name, spec, subdim=False, uops_sha={"v3": sha, "v4": sha})
    dve_ops.OPS.append(op)
    dve_ops.CUSTOM_DVE_SPECS[name] = spec
    return op


def build_module():
    scan_op = _make_scan_op()
    nc = bacc.Bacc("TRN2")
    wdt = mybir.dt.bfloat16 if WW_DT == "bf16" else F32R

    # Host-prepped inputs (all layout transforms only)
    xT0d = nc.dram_tensor("xT0", [128, BN], wdt, kind="ExternalInput")
    xT1d = nc.dram_tensor("xT1", [STATE - 128, BN], wdt, kind="ExternalInput")
    idsTd = nc.dram_tensor("idsT", [ID + 1, BN], F32R, kind="ExternalInput")
    l1WTd = nc.dram_tensor("l1WT", [ID + 1, 10 * E], F32R, kind="ExternalInput")
    wWs, wbs, bTWs, bbs = [], [], [], []
    for i, (S, M, _) in enumerate(LAYERS, start=1):
        wWs.append(nc.dram_tensor(f"w{i}W", [S, M * E], wdt, kind="ExternalInput"))
        wbs.append(nc.dram_tensor(f"w{i}b", [S, M], wdt, kind="ExternalInput"))
        bTWs.append(nc.dram_tensor(f"bT{i}W", [E, M], F32R, kind="ExternalInput"))
        bbs.append(nc.dram_tensor(f"b{i}b", [1, M], F32R, kind="ExternalInput"))
    out = nc.dram_tensor("out", [BN, A], F32, kind="ExternalOutput")

    with ExitStack() as ctx:
        tc = ctx.enter_context(tile.TileContext(nc))
        const1 = ctx.enter_context(tc.tile_pool(name="const1", bufs=1))
        xpool = ctx.enter_context(tc.tile_pool(name="xpool", bufs=10))
        wpool = ctx.enter_context(tc.tile_pool(name="wpool", bufs=WPOOL_BUFS))
        lcpool = ctx.enter_context(tc.tile_pool(name="lcpool", bufs=2))
        ypool = ctx.enter_context(tc.tile_pool(name="ypool", bufs=4))
        tpool = ctx.enter_context(tc.tile_pool(name="tpool", bufs=6))
        spool = ctx.enter_context(tc.tile_pool(name="spool", bufs=4))
        hpool = (None if HREP_BCAST else
                 ctx.enter_context(tc.tile_pool(name="hpool", bufs=10)))
        opool = ctx.enter_context(tc.tile_pool(name="opool", bufs=2))
        pc = ctx.enter_context(tc.tile_pool(name="pc", bufs=3, space="PSUM"))
        pbase = ctx.enter_context(tc.tile_pool(name="pbase", bufs=1, space="PSUM"))
        pmisc = ctx.enter_context(tc.tile_pool(name="pmisc", bufs=1, space="PSUM"))

        import contextlib
        loop_cm = tc.For_i(0, REPEAT, 1) if REPEAT > 1 else contextlib.nullcontext()
        with loop_cm:
            ident = const1.tile([128, 128], F32, tag="ident")
            from concourse.masks import make_identity
            make_identity(nc, ident)
            ones_f32 = const1.tile([1, BN], F32, tag="ones_f")
            nc.gpsimd.memset(ones_f32, 1.0)
            ones_row = const1.tile([1, BN], F32R, tag="ones")
            nc.gpsimd.tensor_copy(ones_row, ones_f32)

            # hypernet operands first (critical path), then layer-1 stationaries
            idsT = const1.tile([ID + 1, BN], F32R, tag="idsT")
            nc.sync.dma_start(idsT, idsTd[:, :])
            l1WT = const1.tile([ID + 1, 10 * E], F32R, tag="l1WT")
            nc.sync.dma_start(l1WT, l1WTd[:, :])
            xT0t = const1.tile([128, BN], wdt, tag="xT0t")
            nc.sync.dma_start(xT0t, xT0d[:, :])
            xT1t = const1.tile([STATE - 128, BN], wdt, tag="xT1t")
            nc.sync.dma_start(xT1t, xT1d[:, :])

            xT = {}
            for bn in range(NBN):
                bsl = slice(bn * 128, (bn + 1) * 128)
                xT[(0, bn)] = xT0t[:, bsl]
                xT[(1, bn)] = xT1t[:, bsl]

            # hypernet: h_iw [bn, 5E] fp32 (relu) and hT_ib [e, BN] per layer (f32r)
            h_iw = []
            for bn in range(NBN):
                hi = const1.tile([128, 5 * E], F32, tag=f"hiw{bn}")
                for j in range(5):
                    iw = 2 * j
                    pm = pmisc.tile([128, 128], F32, tag="pm")
                    nc.tensor.matmul(
                        pm, idsT[:, bn * 128 : (bn + 1) * 128],
                        l1WT[:, iw * E : (iw + 1) * E], start=True, stop=True,
                    )
                    nc.scalar.activation(hi[:, j * E : (j + 1) * E], pm, AF.Relu)
                h_iw.append(hi)
            hT_ib = []
            for j in range(5):
                ib = 2 * j + 1
                pm = pmisc.tile([128, BN], F32, tag="pm")
                nc.tensor.matmul(
                    pm, l1WT[:, ib * E : (ib + 1) * E], idsT, start=True, stop=True
                )
                ht = const1.tile([128, BN], F32R, tag=f"htib{j}")
                nc.scalar.activation(ht, pm, AF.Relu)
                hT_ib.append(ht)

            # scan in1 per (layer, bn): either a stride-0 broadcast view of h
            # ([128, CHUNK//E, E], no SBUF cost) or hrep tiles materialized
            # on ScalarE.
            hrep_all = []
            for li in range(5):
                hl = []
                for bn in range(NBN):
                    if HREP_BCAST:
                        hb = (
                            h_iw[bn][:, li * E : (li + 1) * E]
                            .unsqueeze(1)
                            .broadcast_to([128, CHUNK // E, E])
                        )
                    else:
                        hb = hpool.tile([128, CHUNK], F32, tag="hrep",
                                        name="hrep")
                        for r in range(CHUNK // E):
                            nc.scalar.copy(
                                hb[:, r * E : (r + 1) * E],
                                h_iw[bn][:, li * E : (li + 1) * E],
                            )
                    hl.append(hb)
                hrep_all.append(hl)

            # ---- layers ----
            for li, (S, M, act) in enumerate(LAYERS):
                ME = M * E
                nk = ceil_div(S, 128)
                krows = [min(128, S - k * 128) for k in range(nk)]

                # per-layer constants
                wb_t = []
                for k in range(nk):
                    t = lcpool.tile([128, M], wdt, tag="wb", name="wb")
                    nc.scalar.dma_start(
                        t[: krows[k], :], wbs[li][k * 128 : k * 128 + krows[k], :]
                    )
                    wb_t.append(t)
                bb_t = lcpool.tile([1, M], F32R, tag="bb")
                nc.scalar.dma_start(bb_t, bbs[li][:, :])
                bWT = lcpool.tile([128, M], F32R, tag="bWT")
                nc.scalar.dma_start(bWT, bTWs[li][:, :])

                hrep = hrep_all[li]

                # y_base [bn, M]: x@wb + h_ib@bW.T + 1*bb (psum, evacuated to sbuf)
                pb = []
                for bn in range(NBN) if PROBE != "dmaonly" else []:
                    p = pbase.tile([128, M], F32, tag="pb", name="pb")
                    bsl = slice(bn * 128, (bn + 1) * 128)
                    nc.tensor.matmul(p, xT[(0, bn)][: krows[0], :],
                                     wb_t[0][: krows[0], :], start=True, stop=False)
                    if nk > 1:
                        nc.tensor.matmul(p, xT[(1, bn)][: krows[1], :],
                                         wb_t[1][: krows[1], :], start=False, stop=False)
                    nc.tensor.matmul(p, hT_ib[li][:, bsl], bWT, start=False, stop=False)
                    nc.tensor.matmul(p, ones_row[:, :128], bb_t, start=False, stop=True)
                    psb = ypool.tile([128, M], F32, tag="pbs", name="pbs")
                    nc.scalar.copy(psb, p)
                    pb.append(psb)

                y_sb = []
                for bn in range(NBN):
                    y_sb.append(ypool.tile([128, M], F32, tag="ysb", name="ysb"))
                    if PROBE == "noscan":
                        nc.gpsimd.memset(y_sb[bn][:, :], 0.0)

                # main: C chunks on TensorE; fused scan + boundary diffs on VectorE
                mg = CHUNK // E
                # weight-DMA segments: a couple of CHUNK-sized tiles first (fast
                # pipeline fill), then WDMA-sized
                segs = []
                off = 0
                while off < ME:
                    w_ = CHUNK if (off < 2 * CHUNK and ME > 2 * WDMA) else min(
                        WDMA, ME - off)
                    segs.append((off, w_))
                    off += w_

                def emit_tail_half(k):
                    ksl = slice(k * 128, (k + 1) * 128)
                    for bn in range(NBN):
                        if li < 4:
                            y2 = ypool.tile([128, 128], F32, tag="y2", name="y2")
                            nc.gpsimd.tensor_tensor(out=y2, in0=y_sb[bn][:, ksl],
                                                    in1=pb[bn][:, ksl], op=ALU.add)
                            pt = pmisc.tile([128, 128], F32, tag="pm", name="pm")
                            nc.tensor.transpose(pt, y2, ident)
                            xt = xpool.tile([128, 128], wdt, tag="xt", name="xt")
                            nc.scalar.activation(xt, pt, AF.Relu if act else AF.Copy)
                            xT_next[(k, bn)] = xt
                        else:
                            y2 = ypool.tile([128, A], F32, tag="y2l", name="y2l")
                            nc.gpsimd.tensor_tensor(out=y2, in0=y_sb[bn],
                                                    in1=pb[bn], op=ALU.add)
                            ot = opool.tile([128, A], F32, tag="ot", name="ot")
                            nc.scalar.activation(ot, y2, AF.Relu)
                            nc.gpsimd.dma_start(out[bn * 128 : (bn + 1) * 128, :], ot)

                xT_next = {}
                nhalf = ceil_div(LAYERS[li + 1][0], 128) if li < 4 else 1
                emitted_halves = 0
                cols_done = 0
                chunk_idx = 0
                for off, w_ in segs:
                    wt = []
                    for k in range(nk):
                        t = wpool.tile([128, w_], wdt, tag="w", name="w",
                                       padded_shape=[128, WDMA])
                        dma_eng = nc.sync if k == 0 else nc.scalar
                        dma_eng.dma_start(
                            t[: krows[k], :w_],
                            wWs[li][k * 128 : k * 128 + krows[k], off : off + w_],
                        )
                        wt.append(t)
                    if PROBE == "dmaonly":
                        continue
                    # fp32 PSUM out caps matmuls at 512 elems on TRN2
                    # (walrus s3d3_mm_num_elements rejects 1024-wide).
                    msub = 512
                    for c2 in range(w_ // CHUNK):
                        c = (off + c2 * CHUNK) // CHUNK
                        for bn in range(NBN):
                            p = pc.tile([128, CHUNK], F32, tag="pc", name="pc")
                            for k in range(nk):
                                for sub in range(CHUNK // msub):
                                    ssl = slice(c2 * CHUNK + sub * msub,
                                                c2 * CHUNK + (sub + 1) * msub)
                                    nc.tensor.matmul(
                                        p[:, sub * msub : (sub + 1) * msub],
                                        xT[(k, bn)][: krows[k], :],
                                        wt[k][: krows[k], ssl],
                                        start=(k == 0), stop=(k == nk - 1))
                            if PROBE == "noscan":
                                continue
                            use_pool = (
                                OFFLOAD_MOD and li < 4
                                and chunk_idx % OFFLOAD_MOD == 1
                            )
                            chunk_idx += 1
                            if use_pool:
                                # ACT evacuates PSUM -> SBUF; Pool multiplies
                                # by h and tree-reduces each E-segment.
                                td = spool.tile([128, CHUNK], F32, tag="td",
                                                name="td")
                                nc.scalar.copy(td, p)
                                tm = spool.tile([128, CHUNK], F32, tag="tm",
                                                name="tm")
                                nc.gpsimd.tensor_tensor(
                                    out=tm, in0=td, in1=hrep[bn], op=ALU.mult)
                                tm3 = tm.rearrange("p (s e) -> p s e", e=E)
                                wdh = E // 2
                                while wdh >= 1:
                                    if wdh == 1:
                                        o = y_sb[bn][:, c * mg : (c + 1) * mg]
                                    else:
                                        o = tm3[:, :, :wdh]
                                    nc.gpsimd.tensor_tensor(
                                        out=o, in0=tm3[:, :, :wdh],
                                        in1=tm3[:, :, wdh : 2 * wdh],
                                        op=ALU.add)
                                    wdh //= 2
                                continue
                            t = tpool.tile([128, CHUNK], F32, tag="t", name="t")
                            nc.vector._custom_dve(scan_op, out=t, in0=p, in1=hrep[bn])
                            t3 = t.rearrange("p (s e) -> p s e", e=E)
                            beng = (nc.gpsimd if BOUNDARY_ENG == "gpsimd"
                                    else nc.vector)
                            beng.tensor_copy(
                                y_sb[bn][:, c * mg : c * mg + 1], t[:, E - 1 : E]
                            )
                            beng.tensor_tensor(
                                out=y_sb[bn][:, c * mg + 1 : (c + 1) * mg],
                                in0=t3[:, 1:mg, E - 1],
                                in1=t3[:, 0 : mg - 1, E - 1],
                                op=ALU.subtract,
                            )
                    cols_done = off + w_
                    while (emitted_halves < nhalf
                           and cols_done >= (emitted_halves + 1) * 128 * E
                           and li < 4 and PROBE != "dmaonly"):
                        emit_tail_half(emitted_halves)
                        emitted_halves += 1
                if PROBE == "dmaonly":
                    continue
                if li < 4:
                    while emitted_halves < nhalf:
                        emit_tail_half(emitted_halves)
                        emitted_halves += 1
                    xT = dict(xT_next)
                else:
                    emit_tail_half(0)

    nc.compile()
    return nc


_CACHE = {}


def _get_module():
    if "nc" not in _CACHE:
        _CACHE["nc"] = build_module()
    return _CACHE["nc"]


def _build_in_maps(inputs):
    f = np.float32
    if WW_DT == "bf16":
        import ml_dtypes
        wnp = ml_dtypes.bfloat16
    else:
        wnp = f
    input_full = np.ascontiguousarray(inputs["input"], dtype=f)
    per_core = B // NCORES
    w = {k: np.asarray(v, dtype=f) for k, v in inputs.items() if k != "input"}

    l1WT = np.ascontiguousarray(
        np.vstack([w["l1_W"].T, w["l1_b"][None, :]])
    )
    shared = {"l1WT": l1WT}
    for i in range(1, 6):
        S, M = LAYERS[i - 1][0], LAYERS[i - 1][1]
        shared[f"w{i}W"] = np.ascontiguousarray(
            w[f"w{i}W"].reshape(S, M * E)).astype(wnp)
        shared[f"w{i}b"] = np.ascontiguousarray(
            w[f"w{i}b"].reshape(S, M)).astype(wnp)
        shared[f"bT{i}W"] = np.ascontiguousarray(w[f"b{i}W"].T)
        shared[f"b{i}b"] = np.ascontiguousarray(w[f"b{i}b"].reshape(1, M))

    in_maps = []
    ones = np.ones((1, BN), dtype=f)
    for c in range(NCORES):
        shard = input_full[c * per_core : (c + 1) * per_core].reshape(BN, OBS + OBSU)
        xT_full = np.ascontiguousarray(shard[:, :STATE].T)
        m = dict(shared)
        m["xT0"] = np.ascontiguousarray(xT_full[:128]).astype(wnp)
        m["xT1"] = np.ascontiguousarray(xT_full[128:STATE]).astype(wnp)
        m["idsT"] = np.ascontiguousarray(
            np.vstack([shard[:, STATE:].T, ones])
        )
        in_maps.append(m)
    return in_maps


def _run(inputs, **kw):
    nc = _get_module()
    in_maps = _build_in_maps(inputs)
    res = run_bass_kernel_spmd(nc, in_maps, core_ids=list(range(NCORES)), **kw)
    outs = [r["out"] for r in res.results]
    full = np.concatenate(outs, axis=0).reshape(B, N, A).astype(np.float32)
    return full, res


def kernel(**inputs) -> np.ndarray:
    out, _ = _run(inputs)
    return out

